# revision 19
# baseline (speedup 1.0000x reference)
"""Additive (Bahdanau) attention fused Trainium2 kernel, 8-core data-parallel.

Reference computation (per batch b):
  qp = queries @ W_q            [Q, H]
  kp = keys @ W_k               [K, H]
  scores[q, k] = sum_h w_v[h] * tanh(qp[q, h] + kp[k, h])
  out = softmax_k(scores) @ values

Shapes: B=4, Q=K=1024, D=256, H=64.  Sharding: batch x query-half -> 8 cores
(each core: 512 queries against all 1024 keys of its batch; no collectives).

Key algorithmic trick (vs the tanh-streaming baseline): tanh(x) is
approximated by a short optimized sine series tanh(x) ~= sum_m c_m sin(w_m x),
and sin(w(a+b)) = sin(wa)cos(wb) + cos(wa)sin(wb) turns the whole score
tensor into a single matmul over 2*M*H fp16 sin/cos features:

  score[q,k] ~= sum_{m,h} [c_m w_v[h] sin(w_m qp)] cos(w_m kp)
              + [c_m w_v[h] cos(w_m qp)] sin(w_m kp)

This eliminates the B*Q*K*H tanh stream (the baseline's ScalarE bottleneck,
~190us/core) entirely.  Per core and per frequency m:
  - range reduction to the ScalarE Sin's [-pi,pi] domain via the fp32
    round-to-int magic-constant trick (verified bit-exact on HW):
      t = nu_m*x + phase   (phase 0 / 0.25 turns on partition halves = sin/cos)
      n = (t + 1.5*2^23) - 1.5*2^23     (dual-op tensor_scalar, write rounds)
      v = n - t in [-0.5, 0.5]
    DVE handles the 512 query columns, GPSIMD the 1024 key columns.
  - ScalarE Sin(2*pi*v) -> fp16 features (negation folded: both sides come out
    negated, signs cancel in the product).  HW Sin is f32-exact on [-pi,pi].
  - U features scaled by c_m*w_v[h] (DVE fp16 2x), then PE accumulates
    score chunks [128q, 1024k] in PSUM over m (fp16 matmul, 1 cy/row).
Softmax (constant-shift exp with accum row-sum, reciprocal off-ramp) and
attn @ values follow the baseline: PE transposes of exp-scores, fp16 AV
matmuls, 1/rowsum folded into the output scale.

PSUM (8 banks): one bufs=3 ring of 2-bank slots cycles Xq/Xk projections ->
4 score chunks -> 4 transpose tiles (chunk c3's matmuls deferred until exp(c1)
frees a slot); plus a bufs=2 AV pool.  End-to-end rel err ~3e-3 (gate 2e-2).
"""

import os
import sys

for _p in ("/opt/trn_rl_repo", "/root/.axon_site/_ro/trn_rl_repo"):
    if os.path.isdir(_p) and _p not in sys.path:
        sys.path.append(_p)

import numpy as np

import concourse.bass as bass
import concourse.mybir as mybir
import concourse.tile as tile
from concourse.bass_utils import run_bass_kernel_spmd
from concourse.masks import make_identity
from concourse.vector_clock import ScopedClock

F32 = mybir.dt.float32
FP16 = mybir.dt.float16
AF = mybir.ActivationFunctionType
ALU = mybir.AluOpType

B, Q, K, D, H = 4, 1024, 1024, 256, 64
QC = 512          # queries per core
N_CORES = 8
P = 128           # partitions
TWO_PI = float(2 * np.pi)
CMAGIC = float(1.5 * 2 ** 23)

# optimized sine-series fits of tanh on ~N(0, 1.42^2)-weighted [-11.5, 11.5]
# (least-squares with floor 1e-2; see exp/fit2.py).  MAXAPPROX bounds
# max_x |sum_m c_m sin(w_m x)| for the softmax shift.
FITS = {
    6: ([0.24385319, 0.73424519, 1.23254793, 1.73971083, 2.24761598,
         2.86931623],
        [1.24403698, 0.3458355, 0.14792408, 0.06793787, 0.02801591,
         0.01999234], 1.0247),
    7: ([0.24179256, 0.72806168, 1.22128531, 1.72448478, 2.23721828,
         2.75007586, 3.37702957],
        [1.24430511, 0.34707361, 0.14937783, 0.06788715, 0.03130956,
         0.01287081, 0.00903506], 1.0114),
    8: ([0.23995807, 0.72243414, 1.21167586, 1.70989664, 2.21848689,
         2.73644289, 3.25373744, 3.8853901],
        [1.24473431, 0.34799841, 0.15051655, 0.06890063, 0.03145835,
         0.01441419, 0.00589593, 0.00407801], 1.0054),
}

KM = int(os.environ.get("KM", "8"))
OMEGAS, COEFFS, MAXAPPROX = FITS[KM]
NUS = [float(w / (2 * np.pi)) for w in OMEGAS]
M = KM


def _patched_drain_and_barrier(self, tick_clock, wait_clock):
    """Work around walrus 'Too many sync wait commands': split the kernel-tail
    drain's sem waits so no single instruction carries more than one."""
    drain_inst = self.nc.sync.drain()
    wait_clock.add_sem_waits(
        drain_inst.ins, ScopedClock({None: tick_clock.global_clock})
    )
    si = drain_inst.ins.sync_info
    if si is not None and si.on_wait and len(si.on_wait) > 1:
        waits = list(si.on_wait)
        drain_inst.ins.sync_info = mybir.SyncInfo(
            on_wait=[waits[0]], on_update=list(si.on_update or [])
        )
        for w in waits[1:]:
            extra = self.nc.sync.drain()
            extra.ins.sync_info = mybir.SyncInfo(on_wait=[w], on_update=[])
    self.nc.all_engine_barrier()
    popped = self.nc._tile_sem_poison_stack.pop()
    assert popped is self._sem_poison
    self.nc.clear_and_free_semaphores(list(self.sems.allocated().values()))
    self.nc.all_engine_barrier()


tile.TileContext._drain_and_barrier = _patched_drain_and_barrier

# This walrus build rejects instructions carrying more than one sync-wait
# ("Too many sync wait commands"). Hoist extra waits onto NOPs inserted just
# before the instruction in its engine's stream — semantically identical
# blocking behavior.
MAX_SYNC_WAITS = int(os.environ.get("KMAXW", "1"))


def _split_excess_waits(nc: bass.Bass):
    ctr = 0
    for f in nc.m.functions:
        for bb in f.blocks:
            needs_fix = any(
                getattr(ins, "sync_info", None) is not None
                and ins.sync_info.on_wait
                and len(ins.sync_info.on_wait) > MAX_SYNC_WAITS
                for ins in bb.instructions
            )
            if not needs_fix:
                continue
            new_list = []
            for ins in bb.instructions:
                si = getattr(ins, "sync_info", None)
                if si is not None and si.on_wait and len(si.on_wait) > MAX_SYNC_WAITS:
                    waits = list(si.on_wait)
                    for w in waits[MAX_SYNC_WAITS:]:
                        ctr += 1
                        nop = mybir.InstNoOp(name=f"WS-{ctr}", ins=[], outs=[])
                        nop.engine = ins.engine
                        nop.sync_info = mybir.SyncInfo(on_wait=[w], on_update=[])
                        new_list.append(nop)
                    ins.sync_info = mybir.SyncInfo(
                        on_wait=waits[:MAX_SYNC_WAITS],
                        on_update=list(si.on_update or []),
                    )
                new_list.append(ins)
            bb.instructions = new_list


def build_program(split_waits: bool = True) -> bass.Bass:
    repeat = int(os.environ.get("KREPEAT", "1"))
    # engine split for the range-reduction chains: DVE takes feature columns
    # [0, 512+extra), GPSIMD takes [512+extra, 1536) (q block is 512 wide,
    # k block 1024).  KEXTRA=1024 puts everything on DVE.
    extra = int(os.environ.get("KEXTRA", "0"))
    attn_mix = os.environ.get("KATTN", "mix") == "mix"
    ka_per_m = int(os.environ.get("KKA", "1"))
    use_gps = int(os.environ.get("KGPS", "1"))
    i4_act = int(os.environ.get("KI4ACT", "1"))
    xk_act = int(os.environ.get("KXKACT", "1"))
    warm_n = int(os.environ.get("KWARM", "12"))
    FW = QC + K               # feature width: 512 q-cols | 1024 k-cols
    SPLIT = 512 + extra

    nc = bass.Bass()
    queriesT = nc.declare_dram_parameter("queriesT", [D, QC], FP16, isOutput=False)
    keysT = nc.declare_dram_parameter("keysT", [D, K], FP16, isOutput=False)
    values = nc.declare_dram_parameter("values", [K, D], FP16, isOutput=False)
    Wqdup = nc.declare_dram_parameter("Wqdup", [D, P], FP16, isOutput=False)
    Wkdup = nc.declare_dram_parameter("Wkdup", [D, P], FP16, isOutput=False)
    uscale = nc.declare_dram_parameter("uscale", [P, M], F32, isOutput=False)
    shift = nc.declare_dram_parameter("shift", [P, 1], F32, isOutput=False)
    out = nc.declare_dram_parameter("out", [QC, D], F32, isOutput=True)

    with tile.TileContext(nc) as tc:
        with (
            tc.tile_pool(name="const", bufs=1) as const,
            tc.tile_pool(name="xsb", bufs=1) as xsbp,
            tc.tile_pool(name="tq", bufs=2) as tqp,
            tc.tile_pool(name="tk", bufs=2) as tkp,
            tc.tile_pool(name="vv", bufs=2) as vvp,
            tc.tile_pool(name="feat", bufs=M) as featp,
            tc.tile_pool(name="uscl", bufs=M) as usclp,
            tc.tile_pool(name="expos", bufs=2) as expp,
            tc.tile_pool(name="attns", bufs=2) as attp,
            tc.tile_pool(name="outs", bufs=2) as outp,
            tc.tile_pool(name="stats", bufs=8) as statp,
        ):
            identity = const.tile([P, P], FP16)
            make_identity(nc, identity)

            cvec = const.tile([P, 1], F32)
            nc.vector.memset(cvec[0:64, :], 0.0)
            nc.vector.memset(cvec[64:128, :], 0.25)
            # k-side phases swapped: U.V = sin_q cos_k + cos_q sin_k
            cveck = const.tile([P, 1], F32)
            nc.vector.memset(cveck[0:64, :], 0.25)
            nc.vector.memset(cveck[64:128, :], 0.0)
            shift_sb = const.tile([P, 1], F32)
            uscale_sb = const.tile([P, M], F32)
            values_sb = const.tile([P, 8 * D], FP16)
            Wq_sb = const.tile([P, 2 * P], FP16)
            Wk_sb = const.tile([P, 2 * P], FP16)
            qT = const.tile([P, 2 * QC], FP16)
            kT = const.tile([P, 2 * K], FP16)

            # ---- prologue DMAs (weights first: they gate the projections)
            nc.sync.dma_start(
                Wq_sb[:].rearrange("p (dc w) -> p dc w", dc=2),
                Wqdup[:, :].rearrange("(dc p) w -> p dc w", dc=2),
            )
            nc.sync.dma_start(
                Wk_sb[:].rearrange("p (dc w) -> p dc w", dc=2),
                Wkdup[:, :].rearrange("(dc p) w -> p dc w", dc=2),
            )
            nc.sync.dma_start(
                qT[:].rearrange("p (dc q) -> p dc q", dc=2),
                queriesT[:, :].rearrange("(dc p) q -> p dc q", dc=2),
            )
            nc.sync.dma_start(
                kT[:].rearrange("p (dc k) -> p dc k", dc=2),
                keysT[:, :].rearrange("(dc p) k -> p dc k", dc=2),
            )
            nc.sync.dma_start(uscale_sb, uscale[:, :])
            nc.sync.dma_start(shift_sb, shift[:, :])
            nc.sync.dma_start(
                values_sb[:].rearrange("p (kc v) -> p kc v", kc=8),
                values[:, :].rearrange("(kc p) v -> p kc v", kc=8),
            )
            # preload the trig act table during the DMA wait
            trig_warm = const.tile([P, 1], FP16)
            nc.scalar.activation(trig_warm, cvec, AF.Sin, scale=TWO_PI)

            with tc.tile_pool(name="warm", bufs=1, space="PSUM") as warmp:
                wt = warmp.tile([P, P], F32)
                for _ in range(warm_n):
                    nc.tensor.matmul(
                        wt, Wq_sb[:, 0:P], Wq_sb[:, 0:P], start=True, stop=True
                    )

            def main_body():
                # ---- projections into the big PSUM ring ----
                Xq = bigp.tile([P, K], F32, tag="big")   # use [:, 0:512]
                for dc in range(2):
                    nc.tensor.matmul(
                        Xq[:, 0:QC],
                        Wq_sb[:, dc * P : (dc + 1) * P],
                        qT[:, dc * QC : (dc + 1) * QC],
                        start=(dc == 0),
                        stop=(dc == 1),
                    )
                Xk = bigp.tile([P, K], F32, tag="big")
                for kh in range(2):
                    for dc in range(2):
                        nc.tensor.matmul(
                            Xk[:, kh * 512 : (kh + 1) * 512],
                            Wk_sb[:, dc * P : (dc + 1) * P],
                            kT[:, dc * K + kh * 512 : dc * K + (kh + 1) * 512],
                            start=(dc == 0),
                            stop=(dc == 1),
                        )
                X = xsbp.tile([P, FW], F32, tag="X")
                nc.vector.tensor_copy(X[:, 0:QC], Xq[:, 0:QC])
                if xk_act:
                    nc.scalar.copy(X[:, QC:FW], Xk[:, 0:K])
                else:
                    nc.vector.tensor_copy(X[:, QC:FW], Xk[:, 0:K])

                ps = [
                    bigp.tile([P, K], F32, tag="big", name=f"ps{c}")
                    for c in range(3)
                ]

                # ---- feature pipeline over m; PE accumulates chunks 0-2 ----
                feats, uscs = [], []
                prev_usc = None
                for m in range(M):
                    nu = NUS[m]
                    # GPSIMD: key-side range reduction
                    tk_ = tkp.tile([P, FW - SPLIT], F32, tag="tk")
                    zk = tkp.tile([P, FW - SPLIT], F32, tag="zk")
                    v = vvp.tile([P, FW], F32, tag="v")
                    keng = nc.gpsimd if use_gps else nc.vector
                    keng.tensor_scalar(
                        tk_, X[:, SPLIT:FW], nu, cveck, ALU.mult, ALU.add
                    )
                    keng.tensor_scalar(
                        zk, tk_, CMAGIC, CMAGIC, ALU.add, ALU.subtract
                    )
                    keng.tensor_tensor(
                        v[:, SPLIT:FW], zk, tk_, ALU.subtract
                    )
                    # DVE: query-side range reduction (+ deferred U scale)
                    tq_ = tqp.tile([P, SPLIT], F32, tag="tq")
                    zq = tqp.tile([P, SPLIT], F32, tag="zq")
                    nc.vector.tensor_scalar(
                        tq_, X[:, 0:SPLIT], nu, cvec, ALU.mult, ALU.add
                    )
                    nc.vector.tensor_scalar(
                        zq, tq_, CMAGIC, CMAGIC, ALU.add, ALU.subtract
                    )
                    nc.vector.tensor_tensor(v[:, 0:SPLIT], zq, tq_, ALU.subtract)
                    if prev_usc is not None:
                        # software-pipelined so ACT does sin(m) then i4(m-1)
                        pf, pu, pm = prev_usc
                        if i4_act:
                            nc.scalar.mul(
                                pu, pf[:, 0:QC], uscale_sb[:, pm : pm + 1]
                            )
                        else:
                            nc.vector.tensor_scalar(
                                pu, pf[:, 0:QC], uscale_sb[:, pm : pm + 1],
                                None, ALU.mult,
                            )
                    F_m = featp.tile([P, FW], FP16, tag="F")
                    nc.scalar.activation(F_m, v, AF.Sin, scale=TWO_PI)
                    usc = usclp.tile([P, QC], FP16, tag="usc")
                    feats.append(F_m)
                    uscs.append(usc)
                    prev_usc = (F_m, usc, m)
                    # PE: keepalive + score matmuls for live chunks 0-2
                    for _ in range(ka_per_m):
                        nc.tensor.matmul(
                            wt2, Wq_sb[:, 0:P], qT[:, 0:D],
                            start=True, stop=True,
                        )
                    if m > 0:
                        for c in range(3):
                            for kh in range(2):
                                nc.tensor.matmul(
                                    ps[c][:, kh * 512 : (kh + 1) * 512],
                                    uscs[m - 1][:, c * P : (c + 1) * P],
                                    feats[m - 1][:, QC + kh * 512 : QC + (kh + 1) * 512],
                                    start=(m == 1),
                                    stop=False,
                                )
                pf, pu, pm = prev_usc
                if i4_act:
                    nc.scalar.mul(pu, pf[:, 0:QC], uscale_sb[:, pm : pm + 1])
                else:
                    nc.vector.tensor_scalar(
                        pu, pf[:, 0:QC], uscale_sb[:, pm : pm + 1], None,
                        ALU.mult,
                    )
                for c in range(3):
                    for kh in range(2):
                        nc.tensor.matmul(
                            ps[c][:, kh * 512 : (kh + 1) * 512],
                            uscs[M - 1][:, c * P : (c + 1) * P],
                            feats[M - 1][:, QC + kh * 512 : QC + (kh + 1) * 512],
                            start=(M == 1),
                            stop=True,
                        )

                # ---- per-chunk tails ----
                expt, rsum, rinv, pts, attnT, pav = {}, {}, {}, {}, {}, {}

                def do_exp(c):
                    expt[c] = expp.tile([P, K], FP16, tag="expt", name=f"expt{c}")
                    rsum[c] = statp.tile([P, 1], F32, tag="rsum", name=f"rsum{c}")
                    nc.scalar.activation(
                        expt[c], ps[c], AF.Exp, bias=shift_sb,
                        accum_out=rsum[c],
                    )

                def do_transp(c):
                    pts[c] = bigp.tile([P, K], FP16, tag="big", name=f"pts{c}")
                    for i in range(8):
                        nc.tensor.transpose(
                            pts[c][:, i * P : (i + 1) * P],
                            expt[c][:, i * P : (i + 1) * P],
                            identity,
                        )

                def do_attn_copy(c):
                    attnT[c] = attp.tile([P, K], FP16, tag="attnT", name=f"attnT{c}")
                    if attn_mix and c % 2 == 1:
                        nc.scalar.copy(attnT[c], pts[c])
                    else:
                        nc.vector.tensor_copy(attnT[c], pts[c])

                def do_av(c):
                    pav[c] = pavp.tile([P, D], F32, tag="pav", name=f"pav{c}")
                    for kc in range(8):
                        nc.tensor.matmul(
                            pav[c],
                            attnT[c][:, kc * P : (kc + 1) * P],
                            values_sb[:, kc * D : (kc + 1) * D],
                            start=(kc == 0),
                            stop=(kc == 7),
                        )

                def do_out(c):
                    rinv[c] = statp.tile([P, 1], F32, tag="rinv", name=f"rinv{c}")
                    nc.vector.reciprocal(rinv[c], rsum[c])
                    outt = outp.tile([P, D], F32, tag="outt")
                    nc.vector.tensor_scalar_mul(outt, pav[c], rinv[c])
                    nc.sync.dma_start(out[c * P : (c + 1) * P, :], outt)

                do_exp(0)
                do_exp(1)
                do_transp(0)          # recycles ps0 banks (waits exp 0)
                do_attn_copy(0)
                # chunk 3 scores, deferred into ps1's banks (waits exp 1)
                ps.append(bigp.tile([P, K], F32, tag="big", name="ps3"))
                for m in range(M):
                    for kh in range(2):
                        nc.tensor.matmul(
                            ps[3][:, kh * 512 : (kh + 1) * 512],
                            uscs[m][:, 3 * P : 4 * P],
                            feats[m][:, QC + kh * 512 : QC + (kh + 1) * 512],
                            start=(m == 0),
                            stop=(m == M - 1),
                        )
                do_exp(2)
                do_transp(1)          # recycles ps2 banks (waits exp 2)
                do_attn_copy(1)
                do_av(0)
                do_out(0)
                do_exp(3)
                do_transp(2)          # recycles pt0 banks (waits attnT0 copy)
                do_attn_copy(2)
                do_av(1)
                do_out(1)
                do_transp(3)          # recycles ps3 banks (waits exp 3)
                do_attn_copy(3)
                do_av(2)
                do_out(2)
                do_av(3)
                do_out(3)

            with (
                tc.tile_pool(name="bigp", bufs=3, space="PSUM") as bigp,
                tc.tile_pool(name="pav", bufs=1, space="PSUM") as pavp,
                tc.tile_pool(name="kap", bufs=1, space="PSUM") as kap,
            ):
                wt2 = kap.tile([P, D], F32, tag="ka")
                if repeat == 1:
                    main_body()
                else:
                    with tc.For_i(0, repeat, 1):
                        main_body()

    if split_waits:
        _split_excess_waits(nc)
    return nc


_program_cache = None


def _get_program():
    global _program_cache
    if _program_cache is None:
        _program_cache = build_program()
    return _program_cache


def prep_core_inputs(inputs_np: dict, core: int) -> dict:
    """Host-side (free) prep: slice/transpose/fp16-ify one core's inputs."""
    fp16 = np.float16
    b, qh = divmod(core, 2)
    queries = np.asarray(inputs_np["queries"], np.float32)
    keys = np.asarray(inputs_np["keys"], np.float32)
    values = np.asarray(inputs_np["values"], np.float32)
    W_q = np.asarray(inputs_np["W_q"], np.float32)
    W_k = np.asarray(inputs_np["W_k"], np.float32)
    w_v = np.asarray(inputs_np["w_v"], np.float32).reshape(-1)
    cs = np.asarray(COEFFS, np.float32)
    uscale = (np.concatenate([w_v, w_v])[:, None] * cs[None, :]).astype(
        np.float32
    )
    shift = np.full(
        (P, 1), -float(np.abs(w_v).sum()) * MAXAPPROX * 1.02, dtype=np.float32
    )
    return {
        "queriesT": np.ascontiguousarray(
            queries[b, qh * QC : (qh + 1) * QC, :].T
        ).astype(fp16),
        "keysT": np.ascontiguousarray(keys[b].T).astype(fp16),
        "values": np.ascontiguousarray(values[b]).astype(fp16),
        "Wqdup": np.ascontiguousarray(
            np.concatenate([W_q, W_q], axis=1)
        ).astype(fp16),
        "Wkdup": np.ascontiguousarray(
            np.concatenate([W_k, W_k], axis=1)
        ).astype(fp16),
        "uscale": uscale,
        "shift": shift,
    }


def kernel(queries, keys, values, W_q, W_k, w_v):
    inputs_np = {
        "queries": queries, "keys": keys, "values": values,
        "W_q": W_q, "W_k": W_k, "w_v": w_v,
    }
    nc = _get_program()
    in_maps = [prep_core_inputs(inputs_np, core) for core in range(N_CORES)]
    res = run_bass_kernel_spmd(nc, in_maps, list(range(N_CORES)))
    out = np.empty((B, Q, D), dtype=np.float32)
    for core in range(N_CORES):
        b, qh = divmod(core, 2)
        out[b, qh * QC : (qh + 1) * QC, :] = res.results[core]["out"]
    return out


# revision 20
# speedup vs baseline: 2.8947x; 2.8947x over previous
"""Additive (Bahdanau) attention fused Trainium2 kernel, 8-core data-parallel.

Reference computation (per batch b):
  qp = queries @ W_q            [Q, H]
  kp = keys @ W_k               [K, H]
  scores[q, k] = sum_h w_v[h] * tanh(qp[q, h] + kp[k, h])
  out = softmax_k(scores) @ values

Shapes: B=4, Q=K=1024, D=256, H=64.  Sharding: batch x query-half -> 8 cores
(each core: 512 queries against all 1024 keys of its batch; no collectives).

Key algorithmic trick (vs the tanh-streaming baseline): tanh(x) is
approximated by a short optimized sine series tanh(x) ~= sum_m c_m sin(w_m x),
and sin(w(a+b)) = sin(wa)cos(wb) + cos(wa)sin(wb) turns the whole score
tensor into a single matmul over 2*M*H fp16 sin/cos features:

  score[q,k] ~= sum_{m,h} [c_m w_v[h] sin(w_m qp)] cos(w_m kp)
              + [c_m w_v[h] cos(w_m qp)] sin(w_m kp)

This eliminates the B*Q*K*H tanh stream (the baseline's ScalarE bottleneck,
~190us/core) entirely.  Per core and per frequency m:
  - range reduction to the ScalarE Sin's [-pi,pi] domain via the fp32
    round-to-int magic-constant trick (verified bit-exact on HW):
      t = nu_m*x + phase   (phase 0 / 0.25 turns on partition halves = sin/cos)
      n = (t + 1.5*2^23) - 1.5*2^23     (dual-op tensor_scalar, write rounds)
      v = n - t in [-0.5, 0.5]
    DVE handles the 512 query columns, GPSIMD the 1024 key columns.
  - ScalarE Sin(2*pi*v) -> fp16 features (negation folded: both sides come out
    negated, signs cancel in the product).  HW Sin is f32-exact on [-pi,pi].
  - U features scaled by c_m*w_v[h] (DVE fp16 2x), then PE accumulates
    score chunks [128q, 1024k] in PSUM over m (fp16 matmul, 1 cy/row).
Softmax (constant-shift exp with accum row-sum, reciprocal off-ramp) and
attn @ values follow the baseline: PE transposes of exp-scores, fp16 AV
matmuls, 1/rowsum folded into the output scale.

PSUM (8 banks): one bufs=3 ring of 2-bank slots cycles Xq/Xk projections ->
4 score chunks -> 4 transpose tiles (chunk c3's matmuls deferred until exp(c1)
frees a slot); plus a bufs=2 AV pool.  End-to-end rel err ~3e-3 (gate 2e-2).
"""

import os
import sys

for _p in ("/opt/trn_rl_repo", "/root/.axon_site/_ro/trn_rl_repo"):
    if os.path.isdir(_p) and _p not in sys.path:
        sys.path.append(_p)

import numpy as np

import concourse.bass as bass
import concourse.mybir as mybir
import concourse.tile as tile
from concourse.bass_utils import run_bass_kernel_spmd
from concourse.masks import make_identity
from concourse.vector_clock import ScopedClock

F32 = mybir.dt.float32
FP16 = mybir.dt.float16
AF = mybir.ActivationFunctionType
ALU = mybir.AluOpType

B, Q, K, D, H = 4, 1024, 1024, 256, 64
QC = 512          # queries per core
N_CORES = 8
P = 128           # partitions
TWO_PI = float(2 * np.pi)
CMAGIC = float(1.5 * 2 ** 23)

# optimized sine-series fits of tanh on ~N(0, 1.42^2)-weighted [-11.5, 11.5]
# (least-squares with floor 1e-2; see exp/fit2.py).  MAXAPPROX bounds
# max_x |sum_m c_m sin(w_m x)| for the softmax shift.
FITS = {
    6: ([0.24385319, 0.73424519, 1.23254793, 1.73971083, 2.24761598,
         2.86931623],
        [1.24403698, 0.3458355, 0.14792408, 0.06793787, 0.02801591,
         0.01999234], 1.0247),
    7: ([0.24179256, 0.72806168, 1.22128531, 1.72448478, 2.23721828,
         2.75007586, 3.37702957],
        [1.24430511, 0.34707361, 0.14937783, 0.06788715, 0.03130956,
         0.01287081, 0.00903506], 1.0114),
    8: ([0.23995807, 0.72243414, 1.21167586, 1.70989664, 2.21848689,
         2.73644289, 3.25373744, 3.8853901],
        [1.24473431, 0.34799841, 0.15051655, 0.06890063, 0.03145835,
         0.01441419, 0.00589593, 0.00407801], 1.0054),
}

KM = int(os.environ.get("KM", "8"))
OMEGAS, COEFFS, MAXAPPROX = FITS[KM]
NUS = [float(w / (2 * np.pi)) for w in OMEGAS]
M = KM


def _patched_drain_and_barrier(self, tick_clock, wait_clock):
    """Work around walrus 'Too many sync wait commands': split the kernel-tail
    drain's sem waits so no single instruction carries more than one."""
    drain_inst = self.nc.sync.drain()
    wait_clock.add_sem_waits(
        drain_inst.ins, ScopedClock({None: tick_clock.global_clock})
    )
    si = drain_inst.ins.sync_info
    if si is not None and si.on_wait and len(si.on_wait) > 1:
        waits = list(si.on_wait)
        drain_inst.ins.sync_info = mybir.SyncInfo(
            on_wait=[waits[0]], on_update=list(si.on_update or [])
        )
        for w in waits[1:]:
            extra = self.nc.sync.drain()
            extra.ins.sync_info = mybir.SyncInfo(on_wait=[w], on_update=[])
    self.nc.all_engine_barrier()
    popped = self.nc._tile_sem_poison_stack.pop()
    assert popped is self._sem_poison
    self.nc.clear_and_free_semaphores(list(self.sems.allocated().values()))
    self.nc.all_engine_barrier()


tile.TileContext._drain_and_barrier = _patched_drain_and_barrier

# This walrus build rejects instructions carrying more than one sync-wait
# ("Too many sync wait commands"). Hoist extra waits onto NOPs inserted just
# before the instruction in its engine's stream — semantically identical
# blocking behavior.
MAX_SYNC_WAITS = int(os.environ.get("KMAXW", "1"))


def _split_excess_waits(nc: bass.Bass):
    ctr = 0
    for f in nc.m.functions:
        for bb in f.blocks:
            needs_fix = any(
                getattr(ins, "sync_info", None) is not None
                and ins.sync_info.on_wait
                and len(ins.sync_info.on_wait) > MAX_SYNC_WAITS
                for ins in bb.instructions
            )
            if not needs_fix:
                continue
            new_list = []
            for ins in bb.instructions:
                si = getattr(ins, "sync_info", None)
                if si is not None and si.on_wait and len(si.on_wait) > MAX_SYNC_WAITS:
                    waits = list(si.on_wait)
                    for w in waits[MAX_SYNC_WAITS:]:
                        ctr += 1
                        nop = mybir.InstNoOp(name=f"WS-{ctr}", ins=[], outs=[])
                        nop.engine = ins.engine
                        nop.sync_info = mybir.SyncInfo(on_wait=[w], on_update=[])
                        new_list.append(nop)
                    ins.sync_info = mybir.SyncInfo(
                        on_wait=waits[:MAX_SYNC_WAITS],
                        on_update=list(si.on_update or []),
                    )
                new_list.append(ins)
            bb.instructions = new_list


def build_program(split_waits: bool = True) -> bass.Bass:
    repeat = int(os.environ.get("KREPEAT", "1"))
    # engine split for the range-reduction chains: DVE takes feature columns
    # [0, 512+extra), GPSIMD takes [512+extra, 1536) (q block is 512 wide,
    # k block 1024).  KEXTRA=1024 puts everything on DVE.
    extra = int(os.environ.get("KEXTRA", "0"))
    attn_mix = os.environ.get("KATTN", "mix") == "mix"
    ka_per_m = int(os.environ.get("KKA", "1"))
    use_gps = int(os.environ.get("KGPS", "0"))
    i4_act = int(os.environ.get("KI4ACT", "1"))
    xk_act = int(os.environ.get("KXKACT", "1"))
    warm_n = int(os.environ.get("KWARM", "12"))
    FW = QC + K               # feature width: 512 q-cols | 1024 k-cols
    SPLIT = 512 + extra

    nc = bass.Bass()
    queriesT = nc.declare_dram_parameter("queriesT", [D, QC], FP16, isOutput=False)
    keysT = nc.declare_dram_parameter("keysT", [D, K], FP16, isOutput=False)
    values = nc.declare_dram_parameter("values", [K, D], FP16, isOutput=False)
    Wqdup = nc.declare_dram_parameter("Wqdup", [D, P], FP16, isOutput=False)
    Wkdup = nc.declare_dram_parameter("Wkdup", [D, P], FP16, isOutput=False)
    uscale = nc.declare_dram_parameter("uscale", [P, M], F32, isOutput=False)
    shift = nc.declare_dram_parameter("shift", [P, 1], F32, isOutput=False)
    out = nc.declare_dram_parameter("out", [QC, D], F32, isOutput=True)

    with tile.TileContext(nc) as tc:
        with (
            tc.tile_pool(name="const", bufs=1) as const,
            tc.tile_pool(name="xsb", bufs=1) as xsbp,
            tc.tile_pool(name="tq", bufs=2) as tqp,
            tc.tile_pool(name="tk", bufs=2) as tkp,
            tc.tile_pool(name="vv", bufs=2) as vvp,
            tc.tile_pool(name="feat", bufs=M) as featp,
            tc.tile_pool(name="uscl", bufs=M) as usclp,
            tc.tile_pool(name="expos", bufs=2) as expp,
            tc.tile_pool(name="attns", bufs=2) as attp,
            tc.tile_pool(name="outs", bufs=2) as outp,
            tc.tile_pool(name="stats", bufs=8) as statp,
        ):
            identity = const.tile([P, P], FP16)
            make_identity(nc, identity)

            cvec = const.tile([P, 1], F32)
            nc.vector.memset(cvec[0:64, :], 0.0)
            nc.vector.memset(cvec[64:128, :], 0.25)
            # k-side phases swapped: U.V = sin_q cos_k + cos_q sin_k
            cveck = const.tile([P, 1], F32)
            nc.vector.memset(cveck[0:64, :], 0.25)
            nc.vector.memset(cveck[64:128, :], 0.0)
            shift_sb = const.tile([P, 1], F32)
            uscale_sb = const.tile([P, M], F32)
            values_sb = const.tile([P, 8 * D], FP16)
            Wq_sb = const.tile([P, 2 * P], FP16)
            Wk_sb = const.tile([P, 2 * P], FP16)
            qT = const.tile([P, 2 * QC], FP16)
            kT = const.tile([P, 2 * K], FP16)

            # ---- prologue DMAs (weights first: they gate the projections)
            nc.sync.dma_start(
                Wq_sb[:].rearrange("p (dc w) -> p dc w", dc=2),
                Wqdup[:, :].rearrange("(dc p) w -> p dc w", dc=2),
            )
            nc.sync.dma_start(
                Wk_sb[:].rearrange("p (dc w) -> p dc w", dc=2),
                Wkdup[:, :].rearrange("(dc p) w -> p dc w", dc=2),
            )
            nc.sync.dma_start(
                qT[:].rearrange("p (dc q) -> p dc q", dc=2),
                queriesT[:, :].rearrange("(dc p) q -> p dc q", dc=2),
            )
            nc.sync.dma_start(
                kT[:].rearrange("p (dc k) -> p dc k", dc=2),
                keysT[:, :].rearrange("(dc p) k -> p dc k", dc=2),
            )
            nc.sync.dma_start(uscale_sb, uscale[:, :])
            nc.sync.dma_start(shift_sb, shift[:, :])
            nc.sync.dma_start(
                values_sb[:].rearrange("p (kc v) -> p kc v", kc=8),
                values[:, :].rearrange("(kc p) v -> p kc v", kc=8),
            )
            # preload the trig act table during the DMA wait
            trig_warm = const.tile([P, 1], FP16)
            nc.scalar.activation(trig_warm, cvec, AF.Sin, scale=TWO_PI)

            with tc.tile_pool(name="warm", bufs=1, space="PSUM") as warmp:
                wt = warmp.tile([P, P], F32)
                for _ in range(warm_n):
                    nc.tensor.matmul(
                        wt, Wq_sb[:, 0:P], Wq_sb[:, 0:P], start=True, stop=True
                    )

            def main_body():
                # ---- projections into the big PSUM ring ----
                Xq = bigp.tile([P, K], F32, tag="big")   # use [:, 0:512]
                for dc in range(2):
                    nc.tensor.matmul(
                        Xq[:, 0:QC],
                        Wq_sb[:, dc * P : (dc + 1) * P],
                        qT[:, dc * QC : (dc + 1) * QC],
                        start=(dc == 0),
                        stop=(dc == 1),
                    )
                Xk = bigp.tile([P, K], F32, tag="big")
                for kh in range(2):
                    for dc in range(2):
                        nc.tensor.matmul(
                            Xk[:, kh * 512 : (kh + 1) * 512],
                            Wk_sb[:, dc * P : (dc + 1) * P],
                            kT[:, dc * K + kh * 512 : dc * K + (kh + 1) * 512],
                            start=(dc == 0),
                            stop=(dc == 1),
                        )
                X = xsbp.tile([P, FW], F32, tag="X")
                nc.vector.tensor_copy(X[:, 0:QC], Xq[:, 0:QC])
                if xk_act:
                    nc.scalar.copy(X[:, QC:FW], Xk[:, 0:K])
                else:
                    nc.vector.tensor_copy(X[:, QC:FW], Xk[:, 0:K])

                ps = [
                    bigp.tile([P, K], F32, tag="big", name=f"ps{c}")
                    for c in range(3)
                ]

                # ---- feature pipeline over m; PE accumulates chunks 0-2 ----
                feats, uscs = [], []
                prev_usc = None
                for m in range(M):
                    nu = NUS[m]
                    # GPSIMD: key-side range reduction
                    tk_ = tkp.tile([P, FW - SPLIT], F32, tag="tk")
                    zk = tkp.tile([P, FW - SPLIT], F32, tag="zk")
                    v = vvp.tile([P, FW], F32, tag="v")
                    keng = nc.gpsimd if use_gps else nc.vector
                    keng.tensor_scalar(
                        tk_, X[:, SPLIT:FW], nu, cveck, ALU.mult, ALU.add
                    )
                    keng.tensor_scalar(
                        zk, tk_, CMAGIC, CMAGIC, ALU.add, ALU.subtract
                    )
                    keng.tensor_tensor(
                        v[:, SPLIT:FW], zk, tk_, ALU.subtract
                    )
                    # DVE: query-side range reduction (+ deferred U scale)
                    tq_ = tqp.tile([P, SPLIT], F32, tag="tq")
                    zq = tqp.tile([P, SPLIT], F32, tag="zq")
                    nc.vector.tensor_scalar(
                        tq_, X[:, 0:SPLIT], nu, cvec, ALU.mult, ALU.add
                    )
                    nc.vector.tensor_scalar(
                        zq, tq_, CMAGIC, CMAGIC, ALU.add, ALU.subtract
                    )
                    nc.vector.tensor_tensor(v[:, 0:SPLIT], zq, tq_, ALU.subtract)
                    if prev_usc is not None:
                        # software-pipelined so ACT does sin(m) then i4(m-1)
                        pf, pu, pm = prev_usc
                        if i4_act:
                            nc.scalar.mul(
                                pu, pf[:, 0:QC], uscale_sb[:, pm : pm + 1]
                            )
                        else:
                            nc.vector.tensor_scalar(
                                pu, pf[:, 0:QC], uscale_sb[:, pm : pm + 1],
                                None, ALU.mult,
                            )
                    F_m = featp.tile([P, FW], FP16, tag="F")
                    nc.scalar.activation(F_m, v, AF.Sin, scale=TWO_PI)
                    usc = usclp.tile([P, QC], FP16, tag="usc")
                    feats.append(F_m)
                    uscs.append(usc)
                    prev_usc = (F_m, usc, m)
                    # PE: keepalive + score matmuls for live chunks 0-2
                    for _ in range(ka_per_m):
                        nc.tensor.matmul(
                            wt2, Wq_sb[:, 0:P], qT[:, 0:D],
                            start=True, stop=True,
                        )
                    if m > 0:
                        for c in range(3):
                            for kh in range(2):
                                nc.tensor.matmul(
                                    ps[c][:, kh * 512 : (kh + 1) * 512],
                                    uscs[m - 1][:, c * P : (c + 1) * P],
                                    feats[m - 1][:, QC + kh * 512 : QC + (kh + 1) * 512],
                                    start=(m == 1),
                                    stop=False,
                                )
                pf, pu, pm = prev_usc
                if i4_act:
                    nc.scalar.mul(pu, pf[:, 0:QC], uscale_sb[:, pm : pm + 1])
                else:
                    nc.vector.tensor_scalar(
                        pu, pf[:, 0:QC], uscale_sb[:, pm : pm + 1], None,
                        ALU.mult,
                    )
                for c in range(3):
                    for kh in range(2):
                        nc.tensor.matmul(
                            ps[c][:, kh * 512 : (kh + 1) * 512],
                            uscs[M - 1][:, c * P : (c + 1) * P],
                            feats[M - 1][:, QC + kh * 512 : QC + (kh + 1) * 512],
                            start=(M == 1),
                            stop=True,
                        )

                # ---- per-chunk tails ----
                expt, rsum, rinv, pts, attnT, pav = {}, {}, {}, {}, {}, {}

                def do_exp(c):
                    expt[c] = expp.tile([P, K], FP16, tag="expt", name=f"expt{c}")
                    rsum[c] = statp.tile([P, 1], F32, tag="rsum", name=f"rsum{c}")
                    nc.scalar.activation(
                        expt[c], ps[c], AF.Exp, bias=shift_sb,
                        accum_out=rsum[c],
                    )

                def do_transp(c):
                    pts[c] = bigp.tile([P, K], FP16, tag="big", name=f"pts{c}")
                    for i in range(8):
                        nc.tensor.transpose(
                            pts[c][:, i * P : (i + 1) * P],
                            expt[c][:, i * P : (i + 1) * P],
                            identity,
                        )

                def do_attn_copy(c):
                    attnT[c] = attp.tile([P, K], FP16, tag="attnT", name=f"attnT{c}")
                    if attn_mix and c % 2 == 1:
                        nc.scalar.copy(attnT[c], pts[c])
                    else:
                        nc.vector.tensor_copy(attnT[c], pts[c])

                def do_av(c):
                    pav[c] = pavp.tile([P, D], F32, tag="pav", name=f"pav{c}")
                    for kc in range(8):
                        nc.tensor.matmul(
                            pav[c],
                            attnT[c][:, kc * P : (kc + 1) * P],
                            values_sb[:, kc * D : (kc + 1) * D],
                            start=(kc == 0),
                            stop=(kc == 7),
                        )

                def do_out(c):
                    rinv[c] = statp.tile([P, 1], F32, tag="rinv", name=f"rinv{c}")
                    nc.vector.reciprocal(rinv[c], rsum[c])
                    outt = outp.tile([P, D], F32, tag="outt")
                    nc.vector.tensor_scalar_mul(outt, pav[c], rinv[c])
                    nc.sync.dma_start(out[c * P : (c + 1) * P, :], outt)

                do_exp(0)
                do_exp(1)
                do_transp(0)          # recycles ps0 banks (waits exp 0)
                do_attn_copy(0)
                # chunk 3 scores, deferred into ps1's banks (waits exp 1)
                ps.append(bigp.tile([P, K], F32, tag="big", name="ps3"))
                for m in range(M):
                    for kh in range(2):
                        nc.tensor.matmul(
                            ps[3][:, kh * 512 : (kh + 1) * 512],
                            uscs[m][:, 3 * P : 4 * P],
                            feats[m][:, QC + kh * 512 : QC + (kh + 1) * 512],
                            start=(m == 0),
                            stop=(m == M - 1),
                        )
                do_exp(2)
                do_transp(1)          # recycles ps2 banks (waits exp 2)
                do_attn_copy(1)
                do_av(0)
                do_out(0)
                do_exp(3)
                do_transp(2)          # recycles pt0 banks (waits attnT0 copy)
                do_attn_copy(2)
                do_av(1)
                do_out(1)
                do_transp(3)          # recycles ps3 banks (waits exp 3)
                do_attn_copy(3)
                do_av(2)
                do_out(2)
                do_av(3)
                do_out(3)

            with (
                tc.tile_pool(name="bigp", bufs=3, space="PSUM") as bigp,
                tc.tile_pool(name="pav", bufs=1, space="PSUM") as pavp,
                tc.tile_pool(name="kap", bufs=1, space="PSUM") as kap,
            ):
                wt2 = kap.tile([P, D], F32, tag="ka")
                if repeat == 1:
                    main_body()
                else:
                    with tc.For_i(0, repeat, 1):
                        main_body()

    if split_waits:
        _split_excess_waits(nc)
    return nc


_program_cache = None


def _get_program():
    global _program_cache
    if _program_cache is None:
        _program_cache = build_program()
    return _program_cache


def prep_core_inputs(inputs_np: dict, core: int) -> dict:
    """Host-side (free) prep: slice/transpose/fp16-ify one core's inputs."""
    fp16 = np.float16
    b, qh = divmod(core, 2)
    queries = np.asarray(inputs_np["queries"], np.float32)
    keys = np.asarray(inputs_np["keys"], np.float32)
    values = np.asarray(inputs_np["values"], np.float32)
    W_q = np.asarray(inputs_np["W_q"], np.float32)
    W_k = np.asarray(inputs_np["W_k"], np.float32)
    w_v = np.asarray(inputs_np["w_v"], np.float32).reshape(-1)
    cs = np.asarray(COEFFS, np.float32)
    uscale = (np.concatenate([w_v, w_v])[:, None] * cs[None, :]).astype(
        np.float32
    )
    shift = np.full(
        (P, 1), -float(np.abs(w_v).sum()) * MAXAPPROX * 1.02, dtype=np.float32
    )
    return {
        "queriesT": np.ascontiguousarray(
            queries[b, qh * QC : (qh + 1) * QC, :].T
        ).astype(fp16),
        "keysT": np.ascontiguousarray(keys[b].T).astype(fp16),
        "values": np.ascontiguousarray(values[b]).astype(fp16),
        "Wqdup": np.ascontiguousarray(
            np.concatenate([W_q, W_q], axis=1)
        ).astype(fp16),
        "Wkdup": np.ascontiguousarray(
            np.concatenate([W_k, W_k], axis=1)
        ).astype(fp16),
        "uscale": uscale,
        "shift": shift,
    }


def kernel(queries, keys, values, W_q, W_k, w_v):
    inputs_np = {
        "queries": queries, "keys": keys, "values": values,
        "W_q": W_q, "W_k": W_k, "w_v": w_v,
    }
    nc = _get_program()
    in_maps = [prep_core_inputs(inputs_np, core) for core in range(N_CORES)]
    res = run_bass_kernel_spmd(nc, in_maps, list(range(N_CORES)))
    out = np.empty((B, Q, D), dtype=np.float32)
    for core in range(N_CORES):
        b, qh = divmod(core, 2)
        out[b, qh * QC : (qh + 1) * QC, :] = res.results[core]["out"]
    return out
